# revision 6
# baseline (speedup 1.0000x reference)
"""Trainium2 Bass kernel for nn_MinRNNPredictor (2-layer minGRU + FC head).

Sharding: data-parallel over batch — each of the 8 NeuronCores runs the
full network on one batch row (the recurrence is independent per row);
the small weight matrices are replicated. No collectives.

Per-core dataflow (all on-chip tensors in [feature, time] layout):
  x.T (bf16, via cast-DMA + xbar DMA-transpose)
    -> GEMM0 (PE, bf16 in / fp32 PSUM): pre_z0, pre_h0  [H, Tc]
    -> gates (ScalarE sigmoid, DVE scalar_tensor_tensor)
    -> h0 via DVE TensorTensorScan along the free/time axis
    -> GEMM1 -> gates -> scan -> h1
    -> FC with h1 as the *stationary* operand, producing y in natural
       [time, feature] layout (no output transpose needed).

The time axis is processed in chunks of 512 (one PSUM bank) so the three
GEMM stages of different chunks pipeline on the PE while ACT/DVE run the
gate/scan work of earlier chunks.
"""

import os

# This kernel must run on the axon-tunneled NeuronCores. A host process may
# pin JAX_PLATFORMS=cpu for its own reference math; drop such a pin before
# jax is imported (via concourse) so jax.devices() still sees the cores.
_jp = os.environ.get("JAX_PLATFORMS")
if _jp is not None and "axon" not in _jp and "neuron" not in _jp:
    os.environ.pop("JAX_PLATFORMS", None)

from contextlib import ExitStack

import numpy as np

import concourse.bass as bass
import concourse.mybir as mybir
from concourse import bacc
import concourse.tile as tile
from concourse import bass_utils
from concourse.masks import make_identity

P = 128
B, T, DIN, H, DOUT = 8, 4096, 512, 1024, 512
TC = 512  # time-chunk = one PSUM bank of fp32

F32 = mybir.dt.float32
BF16 = mybir.dt.bfloat16
ALU = mybir.AluOpType
ACTF = mybir.ActivationFunctionType

WEIGHT_NAMES = ("Wz0", "bz0", "Wh0", "bh0", "Wz1", "bz1", "Wh1", "bh1", "Wfc", "bfc")


def _min_gru_layer(nc, sb, psum, w_z, w_h, bz, nbz, bh, rhs_tiles, carry, ltag, c):
    """Emit one time-chunk of one minGRU layer.

    rhs_tiles: K//P SBUF tiles [P, tcc] bf16 with the layer input,
       contraction dim on partitions.
    carry: list of H//P h-tiles from the previous chunk (None for chunk 0);
       updated in place.
    Returns the H//P h tiles [P, tcc] bf16 of this chunk.
    """
    ksub = len(rhs_tiles)
    tcc = rhs_tiles[0].shape[-1]
    h_tiles = []
    for m in range(H // P):
        pz = psum.tile([P, tcc], F32, tag="psum", name=f"pz{ltag}_{c}_{m}")
        ph = psum.tile([P, tcc], F32, tag="psum", name=f"ph{ltag}_{c}_{m}")
        for k in range(ksub):
            nc.tensor.matmul(
                pz[:],
                lhsT=w_z[:, k, m * P : (m + 1) * P],
                rhs=rhs_tiles[k][:],
                start=(k == 0),
                stop=(k == ksub - 1),
            )
        for k in range(ksub):
            nc.tensor.matmul(
                ph[:],
                lhsT=w_h[:, k, m * P : (m + 1) * P],
                rhs=rhs_tiles[k][:],
                start=(k == 0),
                stop=(k == ksub - 1),
            )
        # a = 1 - z = sigmoid(-(pre_z + bz)); z = sigmoid(pre_z + bz)
        a_t = sb.tile([P, tcc], F32, tag=f"a{ltag}", bufs=4, name=f"a{ltag}_{c}_{m}")
        nc.scalar.activation(
            a_t[:], pz[:], ACTF.Sigmoid, bias=nbz[:, m : m + 1], scale=-1.0
        )
        z_t = sb.tile([P, tcc], BF16, tag=f"z{ltag}", bufs=4, name=f"z{ltag}_{c}_{m}")
        nc.scalar.activation(
            z_t[:], pz[:], ACTF.Sigmoid, bias=bz[:, m : m + 1], scale=1.0
        )
        # b = (pre_h + bh) * z
        b_t = sb.tile([P, tcc], F32, tag=f"b{ltag}", bufs=4, name=f"b{ltag}_{c}_{m}")
        nc.vector.scalar_tensor_tensor(
            b_t[:], ph[:], bh[:, m : m + 1], z_t[:], op0=ALU.add, op1=ALU.mult
        )
        # h_t = a_t * h_{t-1} + b_t along the time (free) axis
        h_t = sb.tile(
            [P, tcc], BF16, tag=f"h{ltag}_{m}", bufs=2, name=f"h{ltag}_{c}_{m}"
        )
        init = 0.0 if carry[m] is None else carry[m][:, tcc - 1 : tcc]
        nc.vector.tensor_tensor_scan(
            h_t[:], a_t[:], b_t[:], init, op0=ALU.mult, op1=ALU.add
        )
        carry[m] = h_t
        h_tiles.append(h_t)
    return h_tiles


def build(t_total=T, tcc=TC):
    """Build the single-core Bass module (same NEFF runs SPMD on all cores)."""
    nchunk = t_total // tcc
    assert t_total % tcc == 0 and tcc % P == 0

    nc = bacc.Bacc("TRN2", target_bir_lowering=False, debug=False, num_devices=B)
    x_d = nc.dram_tensor("x", [t_total, DIN], F32, kind="ExternalInput").ap()
    w_d = {}
    for name, shape in (
        ("Wz0", [DIN, H]),
        ("bz0", [H]),
        ("Wh0", [DIN, H]),
        ("bh0", [H]),
        ("Wz1", [H, H]),
        ("bz1", [H]),
        ("Wh1", [H, H]),
        ("bh1", [H]),
        ("Wfc", [H, DOUT]),
        ("bfc", [DOUT]),
    ):
        w_d[name] = nc.dram_tensor(name, shape, F32, kind="ExternalInput").ap()
    y_d = nc.dram_tensor("y", [t_total, DOUT], F32, kind="ExternalOutput").ap()

    with tile.TileContext(nc) as tc, ExitStack() as ctx:
        const = ctx.enter_context(tc.tile_pool(name="const", bufs=1))
        sb = ctx.enter_context(tc.tile_pool(name="sb", bufs=2))
        psum = ctx.enter_context(tc.tile_pool(name="psum", bufs=6, space="PSUM"))
        psum_t = ctx.enter_context(tc.tile_pool(name="psum_t", bufs=2, space="PSUM"))

        # Resident bf16 weights, contraction dim striped onto partitions.
        def load_w(name, k_dim, n_dim):
            t_ = const.tile([P, k_dim // P, n_dim], BF16, name=f"{name}_sb")
            nc.gpsimd.dma_start(
                t_[:], w_d[name].rearrange("(o p) n -> p o n", p=P)
            )
            return t_

        wz0_sb = load_w("Wz0", DIN, H)
        wh0_sb = load_w("Wh0", DIN, H)
        wz1_sb = load_w("Wz1", H, H)
        wh1_sb = load_w("Wh1", H, H)
        wfc_sb = load_w("Wfc", H, DOUT)

        # Gate biases as [P, H//P] fp32 stripes (per-partition scalars).
        def load_bias(name):
            t_ = const.tile([P, H // P], F32, name=f"{name}_sb")
            nc.sync.dma_start(t_[:], w_d[name].rearrange("(o p) -> p o", p=P))
            return t_

        bz0_sb = load_bias("bz0")
        bh0_sb = load_bias("bh0")
        bz1_sb = load_bias("bz1")
        bh1_sb = load_bias("bh1")
        nbz0_sb = const.tile([P, H // P], F32, name="nbz0_sb")
        nc.vector.tensor_scalar_mul(nbz0_sb[:], bz0_sb[:], -1.0)
        nbz1_sb = const.tile([P, H // P], F32, name="nbz1_sb")
        nc.vector.tensor_scalar_mul(nbz1_sb[:], bz1_sb[:], -1.0)

        # FC bias replicated across partitions (free-dim vector).
        bfc_sb = const.tile([P, DOUT], F32, name="bfc_sb")
        nc.sync.dma_start(
            bfc_sb[:],
            w_d["bfc"].rearrange("(a d) -> a d", a=1).to_broadcast((P, DOUT)),
        )

        # Identity for PE-based transposes (fp32 input -> fp32 psum).
        ident = const.tile([P, P], F32, name="ident")
        make_identity(nc, ident[:])

        carry0 = [None] * (H // P)
        carry1 = [None] * (H // P)
        for c in range(nchunk):
            # Load x chunk (HWDGE, fp32), natural [time, feat] layout.
            x_nat = sb.tile(
                [P, tcc // P, DIN], F32, tag="x_nat", bufs=2, name=f"x_nat_{c}"
            )
            nc.sync.dma_start(
                x_nat[:],
                x_d[c * tcc : (c + 1) * tcc, :].rearrange("(o p) d -> p o d", p=P),
            )
            # x.T tiles via PE transpose; the ACT evacuation casts to bf16.
            xT = []
            for dj in range(DIN // P):
                t_ = sb.tile([P, tcc], BF16, tag=f"xT{dj}", bufs=2, name=f"xT{dj}_{c}")
                for ts in range(tcc // P):
                    tp = psum_t.tile([P, P], F32, tag="tpsum", name=f"tp_{c}_{dj}_{ts}")
                    nc.tensor.transpose(
                        tp[:], x_nat[:, ts, dj * P : (dj + 1) * P], ident[:]
                    )
                    nc.scalar.copy(t_[:, ts * P : (ts + 1) * P], tp[:])
                xT.append(t_)

            h0 = _min_gru_layer(
                nc, sb, psum, wz0_sb, wh0_sb, bz0_sb, nbz0_sb, bh0_sb, xT, carry0,
                "0", c,
            )
            h1 = _min_gru_layer(
                nc, sb, psum, wz1_sb, wh1_sb, bz1_sb, nbz1_sb, bh1_sb, h0, carry1,
                "1", c,
            )

            # FC: h1 stationary, Wfc moving -> y in natural [time, feature].
            for tt in range(tcc // P):
                yp = psum.tile([P, DOUT], F32, tag="psum", name=f"yp_{c}_{tt}")
                for j in range(H // P):
                    nc.tensor.matmul(
                        yp[:],
                        lhsT=h1[j][:, tt * P : (tt + 1) * P],
                        rhs=wfc_sb[:, j, :],
                        start=(j == 0),
                        stop=(j == H // P - 1),
                    )
                y_sb = sb.tile([P, DOUT], F32, tag="y", bufs=4, name=f"y_{c}_{tt}")
                nc.vector.tensor_tensor(y_sb[:], yp[:], bfc_sb[:], ALU.add)
                t0 = c * tcc + tt * P
                nc.sync.dma_start(y_d[t0 : t0 + P, :], y_sb[:])

    nc.compile()
    return nc


_NC_CACHE = {}


def _get_nc(t_total=T, tcc=TC):
    key = (t_total, tcc)
    if key not in _NC_CACHE:
        _NC_CACHE[key] = build(t_total, tcc)
    return _NC_CACHE[key]


def run(inputs, trace=False, **spmd_kwargs):
    """Run the SPMD kernel on all 8 cores. Returns (y[B,T,DOUT], results)."""
    x = np.ascontiguousarray(np.asarray(inputs["x"], dtype=np.float32))
    assert x.shape == (B, T, DIN), x.shape
    shared = {
        name: np.ascontiguousarray(np.asarray(inputs[name], dtype=np.float32))
        for name in WEIGHT_NAMES
    }
    nc = _get_nc()
    in_maps = [dict(shared, x=x[c]) for c in range(B)]
    res = bass_utils.run_bass_kernel_spmd(
        nc, in_maps, core_ids=list(range(B)), trace=trace, **spmd_kwargs
    )
    y = np.stack([r["y"] for r in res.results], axis=0).astype(np.float32)
    return y, res


def kernel(**inputs) -> np.ndarray:
    y, _ = run(inputs)
    return y


# revision 7
# speedup vs baseline: 1.1322x; 1.1322x over previous
"""Trainium2 Bass kernel for nn_MinRNNPredictor (2-layer minGRU + FC head).

Sharding: data-parallel over batch — each of the 8 NeuronCores runs the
full network on one batch row (the recurrence is independent per row);
the small weight matrices are replicated. No collectives.

Per-core dataflow (all on-chip tensors in [feature, time] layout):
  x.T (PE transpose of fp32 x, ACT evacuation casts to bf16)
    -> GEMM0 (PE, bf16 in / fp32 PSUM): pre_z0, pre_h0  [H, Tc]
    -> gates (ScalarE sigmoid, DVE scalar_tensor_tensor)
    -> h0 via DVE TensorTensorScan along the free/time axis
    -> GEMM1 -> gates -> scan -> h1
    -> FC with h1 as the *stationary* operand, producing y in natural
       [time, feature] layout (no output transpose needed).

The time axis is processed in chunks of 512 (one PSUM bank). The chunk
loop is software-pipelined: per iteration the PE runs GEMM0(i), the
transposes for chunk i+1, GEMM1(i-1) and FC(i-2), so the serial DVE scan
chain of a chunk always overlaps a full iteration of PE work instead of
stalling the PE at chunk boundaries.
"""

import os

# This kernel must run on the axon-tunneled NeuronCores. A host process may
# pin JAX_PLATFORMS=cpu for its own reference math; drop such a pin before
# jax is imported (via concourse) so jax.devices() still sees the cores.
_jp = os.environ.get("JAX_PLATFORMS")
if _jp is not None and "axon" not in _jp and "neuron" not in _jp:
    os.environ.pop("JAX_PLATFORMS", None)

from contextlib import ExitStack

import numpy as np

import concourse.bass as bass
import concourse.mybir as mybir
import concourse.tile as tile
from concourse import bacc, bass_utils
from concourse.masks import make_identity

P = 128
B, T, DIN, H, DOUT = 8, 4096, 512, 1024, 512
TC = 512  # time-chunk = one PSUM bank of fp32

F32 = mybir.dt.float32
BF16 = mybir.dt.bfloat16
ALU = mybir.AluOpType
ACTF = mybir.ActivationFunctionType

WEIGHT_NAMES = ("Wz0", "bz0", "Wh0", "bh0", "Wz1", "bz1", "Wh1", "bh1", "Wfc", "bfc")


def build(t_total=T, tcc=TC):
    """Build the single-core Bass module (same NEFF runs SPMD on all cores)."""
    nchunk = t_total // tcc
    assert t_total % tcc == 0 and tcc % P == 0
    hsub = H // P

    nc = bacc.Bacc("TRN2", target_bir_lowering=False, debug=False, num_devices=B)
    x_d = nc.dram_tensor("x", [t_total, DIN], F32, kind="ExternalInput").ap()
    w_d = {}
    for name, shape in (
        ("Wz0", [DIN, H]),
        ("bz0", [H]),
        ("Wh0", [DIN, H]),
        ("bh0", [H]),
        ("Wz1", [H, H]),
        ("bz1", [H]),
        ("Wh1", [H, H]),
        ("bh1", [H]),
        ("Wfc", [H, DOUT]),
        ("bfc", [DOUT]),
    ):
        w_d[name] = nc.dram_tensor(name, shape, F32, kind="ExternalInput").ap()
    y_d = nc.dram_tensor("y", [t_total, DOUT], F32, kind="ExternalOutput").ap()

    with tile.TileContext(nc) as tc, ExitStack() as ctx:
        const = ctx.enter_context(tc.tile_pool(name="const", bufs=1))
        sb = ctx.enter_context(tc.tile_pool(name="sb", bufs=2))
        psum = ctx.enter_context(tc.tile_pool(name="psum", bufs=6, space="PSUM"))
        psum_t = ctx.enter_context(tc.tile_pool(name="psum_t", bufs=2, space="PSUM"))

        # Identity for PE transposes FIRST: gpsimd executes in order, so this
        # must precede the big SWDGE weight casts or the first transposes
        # would wait ~40us for them.
        ident = const.tile([P, P], F32, name="ident")
        make_identity(nc, ident[:])

        # Gate biases as [P, H//P] fp32 stripes (per-partition scalars).
        def load_bias(name):
            t_ = const.tile([P, hsub], F32, name=f"{name}_sb")
            nc.sync.dma_start(t_[:], w_d[name].rearrange("(o p) -> p o", p=P))
            return t_

        bz0_sb = load_bias("bz0")
        bh0_sb = load_bias("bh0")
        bz1_sb = load_bias("bz1")
        bh1_sb = load_bias("bh1")
        nbz0_sb = const.tile([P, hsub], F32, name="nbz0_sb")
        nc.vector.tensor_scalar_mul(nbz0_sb[:], bz0_sb[:], -1.0)
        nbz1_sb = const.tile([P, hsub], F32, name="nbz1_sb")
        nc.vector.tensor_scalar_mul(nbz1_sb[:], bz1_sb[:], -1.0)

        # FC bias replicated across partitions (free-dim vector).
        bfc_sb = const.tile([P, DOUT], F32, name="bfc_sb")
        nc.sync.dma_start(
            bfc_sb[:],
            w_d["bfc"].rearrange("(a d) -> a d", a=1).to_broadcast((P, DOUT)),
        )

        # Resident bf16 weights (SWDGE cast-DMA), contraction on partitions.
        # Layer-0 weights first — GEMM0(0) needs them earliest; the rest
        # stream in under the first chunk's compute.
        def load_w(name, k_dim, n_dim):
            t_ = const.tile([P, k_dim // P, n_dim], BF16, name=f"{name}_sb")
            nc.gpsimd.dma_start(t_[:], w_d[name].rearrange("(o p) n -> p o n", p=P))
            return t_

        wz0_sb = load_w("Wz0", DIN, H)
        wh0_sb = load_w("Wh0", DIN, H)
        wz1_sb = load_w("Wz1", H, H)
        wh1_sb = load_w("Wh1", H, H)
        wfc_sb = load_w("Wfc", H, DOUT)

        xT_tiles = {}
        h0_tiles = {}
        h1_tiles = {}
        carry0 = [None] * hsub
        carry1 = [None] * hsub

        def emit_T(i):
            """Load x chunk i (fp32) and produce bf16 x.T tiles via PE."""
            x_nat = sb.tile(
                [P, tcc // P, DIN], F32, tag="x_nat", bufs=2, name=f"x_nat_{i}"
            )
            nc.sync.dma_start(
                x_nat[:],
                x_d[i * tcc : (i + 1) * tcc, :].rearrange("(o p) d -> p o d", p=P),
            )
            xT = []
            for dj in range(DIN // P):
                t_ = sb.tile([P, tcc], BF16, tag=f"xT{dj}", bufs=2, name=f"xT{dj}_{i}")
                for ts in range(tcc // P):
                    tp = psum_t.tile([P, P], F32, tag="tpsum", name=f"tp_{i}_{dj}_{ts}")
                    nc.tensor.transpose(
                        tp[:], x_nat[:, ts, dj * P : (dj + 1) * P], ident[:]
                    )
                    nc.scalar.copy(t_[:, ts * P : (ts + 1) * P], tp[:])
                xT.append(t_)
            xT_tiles[i] = xT

        def emit_layer(i, w_z, w_h, bz, nbz, bh, rhs_tiles, carry, out_tiles, ltag):
            ksub = len(rhs_tiles)
            outs = []
            for m in range(hsub):
                pz = psum.tile([P, tcc], F32, tag="psum", name=f"pz{ltag}_{i}_{m}")
                ph = psum.tile([P, tcc], F32, tag="psum", name=f"ph{ltag}_{i}_{m}")
                for k in range(ksub):
                    nc.tensor.matmul(
                        pz[:],
                        lhsT=w_z[:, k, m * P : (m + 1) * P],
                        rhs=rhs_tiles[k][:],
                        start=(k == 0),
                        stop=(k == ksub - 1),
                    )
                for k in range(ksub):
                    nc.tensor.matmul(
                        ph[:],
                        lhsT=w_h[:, k, m * P : (m + 1) * P],
                        rhs=rhs_tiles[k][:],
                        start=(k == 0),
                        stop=(k == ksub - 1),
                    )
                # a = 1 - z = sigmoid(-(pre_z + bz)); z = sigmoid(pre_z + bz)
                a_t = sb.tile(
                    [P, tcc], BF16, tag=f"a{ltag}", bufs=4, name=f"a{ltag}_{i}_{m}"
                )
                nc.scalar.activation(
                    a_t[:], pz[:], ACTF.Sigmoid, bias=nbz[:, m : m + 1], scale=-1.0
                )
                z_t = sb.tile(
                    [P, tcc], BF16, tag=f"z{ltag}", bufs=4, name=f"z{ltag}_{i}_{m}"
                )
                nc.scalar.activation(
                    z_t[:], pz[:], ACTF.Sigmoid, bias=bz[:, m : m + 1], scale=1.0
                )
                # b = (pre_h + bh) * z
                b_t = sb.tile(
                    [P, tcc], BF16, tag=f"b{ltag}", bufs=4, name=f"b{ltag}_{i}_{m}"
                )
                nc.vector.scalar_tensor_tensor(
                    b_t[:], ph[:], bh[:, m : m + 1], z_t[:], op0=ALU.add, op1=ALU.mult
                )
                # h_t = a_t * h_{t-1} + b_t along the time (free) axis
                h_t = sb.tile(
                    [P, tcc], BF16, tag=f"h{ltag}_{m}", bufs=3, name=f"h{ltag}_{i}_{m}"
                )
                init = 0.0 if carry[m] is None else carry[m][:, tcc - 1 : tcc]
                nc.vector.tensor_tensor_scan(
                    h_t[:], a_t[:], b_t[:], init, op0=ALU.mult, op1=ALU.add
                )
                carry[m] = h_t
                outs.append(h_t)
            out_tiles[i] = outs

        def emit_FC(i):
            h1 = h1_tiles.pop(i)
            for tt in range(tcc // P):
                yp = psum.tile([P, DOUT], F32, tag="psum", name=f"yp_{i}_{tt}")
                for j in range(hsub):
                    nc.tensor.matmul(
                        yp[:],
                        lhsT=h1[j][:, tt * P : (tt + 1) * P],
                        rhs=wfc_sb[:, j, :],
                        start=(j == 0),
                        stop=(j == hsub - 1),
                    )
                y_sb = sb.tile([P, DOUT], F32, tag="y", bufs=4, name=f"y_{i}_{tt}")
                nc.vector.tensor_tensor(y_sb[:], yp[:], bfc_sb[:], ALU.add)
                t0 = i * tcc + tt * P
                nc.sync.dma_start(y_d[t0 : t0 + P, :], y_sb[:])

        # Software-pipelined chunk loop (stages offset on the PE stream).
        emit_T(0)
        for i in range(nchunk + 2):
            if i < nchunk:
                emit_layer(
                    i, wz0_sb, wh0_sb, bz0_sb, nbz0_sb, bh0_sb,
                    xT_tiles.pop(i), carry0, h0_tiles, "0",
                )
            if i + 1 < nchunk:
                emit_T(i + 1)
            if 1 <= i <= nchunk:
                emit_layer(
                    i - 1, wz1_sb, wh1_sb, bz1_sb, nbz1_sb, bh1_sb,
                    h0_tiles.pop(i - 1), carry1, h1_tiles, "1",
                )
            if 2 <= i <= nchunk + 1:
                emit_FC(i - 2)

    nc.compile()
    return nc


_NC_CACHE = {}


def _get_nc(t_total=T, tcc=TC):
    key = (t_total, tcc)
    if key not in _NC_CACHE:
        _NC_CACHE[key] = build(t_total, tcc)
    return _NC_CACHE[key]


def run(inputs, trace=False, **spmd_kwargs):
    """Run the SPMD kernel on all 8 cores. Returns (y[B,T,DOUT], results)."""
    x = np.ascontiguousarray(np.asarray(inputs["x"], dtype=np.float32))
    assert x.shape == (B, T, DIN), x.shape
    shared = {
        name: np.ascontiguousarray(np.asarray(inputs[name], dtype=np.float32))
        for name in WEIGHT_NAMES
    }
    nc = _get_nc()
    in_maps = [dict(shared, x=x[c]) for c in range(B)]
    res = bass_utils.run_bass_kernel_spmd(
        nc, in_maps, core_ids=list(range(B)), trace=trace, **spmd_kwargs
    )
    y = np.stack([r["y"] for r in res.results], axis=0).astype(np.float32)
    return y, res


def kernel(**inputs) -> np.ndarray:
    y, _ = run(inputs)
    return y


# revision 8
# speedup vs baseline: 1.2242x; 1.0813x over previous
"""Trainium2 Bass kernel for nn_MinRNNPredictor (2-layer minGRU + FC head).

Sharding: data-parallel over batch — each of the 8 NeuronCores runs the
full network on one batch row (the recurrence is independent per row);
the small weight matrices are replicated. No collectives.

Per-core dataflow (all on-chip tensors in [feature, time] layout):
  x.T (bf16, host-cast; xbar DMA-transpose straight from DRAM)
    -> GEMM0 (PE, bf16 in / fp32 PSUM): pre_z0, pre_h0  [H, Tc]
    -> gates (ScalarE sigmoid, DVE scalar_tensor_tensor)
    -> h0 via DVE TensorTensorScan along the free/time axis
    -> GEMM1 -> gates -> scan -> h1
    -> FC with h1 as the *stationary* operand, producing y in natural
       [time, feature] layout (no output transpose needed).

Weights and x are cast to bf16 on the host once (the GEMM operands are
bf16 on-chip either way), halving the weight/x HBM traffic that
otherwise dominates the kernel head.

The time axis is processed in chunks of 512 (one PSUM bank). The chunk
loop is software-pipelined: per iteration the PE runs GEMM0(i),
GEMM1(i-1) and FC(i-2), so the serial DVE scan chain of a chunk always
overlaps a full iteration of PE work instead of stalling the PE at
chunk boundaries.
"""

import os

# This kernel must run on the axon-tunneled NeuronCores. A host process may
# pin JAX_PLATFORMS=cpu for its own reference math; drop such a pin before
# jax is imported (via concourse) so jax.devices() still sees the cores.
_jp = os.environ.get("JAX_PLATFORMS")
if _jp is not None and "axon" not in _jp and "neuron" not in _jp:
    os.environ.pop("JAX_PLATFORMS", None)

from contextlib import ExitStack

import ml_dtypes
import numpy as np

import concourse.bass as bass
import concourse.mybir as mybir
import concourse.tile as tile
from concourse import bacc, bass_utils

P = 128
B, T, DIN, H, DOUT = 8, 4096, 512, 1024, 512
TC = 512  # time-chunk = one PSUM bank of fp32

F32 = mybir.dt.float32
BF16 = mybir.dt.bfloat16
ALU = mybir.AluOpType
ACTF = mybir.ActivationFunctionType

GEMM_W = ("Wz0", "Wh0", "Wz1", "Wh1", "Wfc")
BIASES = ("bz0", "bh0", "bz1", "bh1", "bfc")
WEIGHT_NAMES = ("Wz0", "bz0", "Wh0", "bh0", "Wz1", "bz1", "Wh1", "bh1", "Wfc", "bfc")


def build(t_total=T, tcc=TC):
    """Build the single-core Bass module (same NEFF runs SPMD on all cores)."""
    nchunk = t_total // tcc
    assert t_total % tcc == 0 and tcc % P == 0
    hsub = H // P

    nc = bacc.Bacc("TRN2", target_bir_lowering=False, debug=False, num_devices=B)
    x_d = nc.dram_tensor("x", [t_total, DIN], BF16, kind="ExternalInput").ap()
    w_d = {}
    for name, shape, dt in (
        ("Wz0", [DIN, H], BF16),
        ("bz0", [H], F32),
        ("Wh0", [DIN, H], BF16),
        ("bh0", [H], F32),
        ("Wz1", [H, H], BF16),
        ("bz1", [H], F32),
        ("Wh1", [H, H], BF16),
        ("bh1", [H], F32),
        ("Wfc", [H, DOUT], BF16),
        ("bfc", [DOUT], F32),
    ):
        w_d[name] = nc.dram_tensor(name, shape, dt, kind="ExternalInput").ap()
    y_d = nc.dram_tensor("y", [t_total, DOUT], F32, kind="ExternalOutput").ap()

    with tile.TileContext(nc) as tc, ExitStack() as ctx:
        const = ctx.enter_context(tc.tile_pool(name="const", bufs=1))
        sb = ctx.enter_context(tc.tile_pool(name="sb", bufs=2))
        psum = ctx.enter_context(tc.tile_pool(name="psum", bufs=7, space="PSUM"))

        xT_tiles = {}
        h0_tiles = {}
        h1_tiles = {}
        carry0 = [None] * hsub
        carry1 = [None] * hsub

        def emit_T(i):
            """x.T tiles for chunk i via xbar DMA-transpose from DRAM."""
            xT = []
            for dj in range(DIN // P):
                t_ = sb.tile([P, tcc], BF16, tag=f"xT{dj}", bufs=2, name=f"xT{dj}_{i}")
                nc.sync.dma_start_transpose(
                    t_[:], x_d[i * tcc : (i + 1) * tcc, dj * P : (dj + 1) * P]
                )
                xT.append(t_)
            xT_tiles[i] = xT

        # x chunk 0 first: it gates the very first GEMM.
        emit_T(0)

        # Resident bf16 weights (HWDGE), contraction dim on partitions.
        # Layer-0 weights first — GEMM0(0) needs them earliest; the rest
        # stream in under the first chunks' compute.
        def load_w(name, k_dim, n_dim):
            t_ = const.tile([P, k_dim // P, n_dim], BF16, name=f"{name}_sb")
            nc.sync.dma_start(t_[:], w_d[name].rearrange("(o p) n -> p o n", p=P))
            return t_

        wz0_sb = load_w("Wz0", DIN, H)
        wh0_sb = load_w("Wh0", DIN, H)
        wz1_sb = load_w("Wz1", H, H)
        wh1_sb = load_w("Wh1", H, H)
        wfc_sb = load_w("Wfc", H, DOUT)

        # Gate biases as [P, H//P] fp32 stripes (per-partition scalars).
        def load_bias(name):
            t_ = const.tile([P, hsub], F32, name=f"{name}_sb")
            nc.sync.dma_start(t_[:], w_d[name].rearrange("(o p) -> p o", p=P))
            return t_

        bz0_sb = load_bias("bz0")
        bh0_sb = load_bias("bh0")
        bz1_sb = load_bias("bz1")
        bh1_sb = load_bias("bh1")
        nbz0_sb = const.tile([P, hsub], F32, name="nbz0_sb")
        nc.vector.tensor_scalar_mul(nbz0_sb[:], bz0_sb[:], -1.0)
        nbz1_sb = const.tile([P, hsub], F32, name="nbz1_sb")
        nc.vector.tensor_scalar_mul(nbz1_sb[:], bz1_sb[:], -1.0)

        # FC bias replicated across partitions (free-dim vector). Emitted
        # last: its 128-descriptor broadcast DMA is slow and only needed by
        # FC(0), ~100us into the kernel.
        bfc_sb = const.tile([P, DOUT], F32, name="bfc_sb")
        nc.sync.dma_start(
            bfc_sb[:],
            w_d["bfc"].rearrange("(a d) -> a d", a=1).to_broadcast((P, DOUT)),
        )

        def emit_layer(i, w_z, w_h, bz, nbz, bh, rhs_tiles, carry, out_tiles, ltag):
            ksub = len(rhs_tiles)
            outs = []
            for m in range(hsub):
                pz = psum.tile([P, tcc], F32, tag="psum", name=f"pz{ltag}_{i}_{m}")
                ph = psum.tile([P, tcc], F32, tag="psum", name=f"ph{ltag}_{i}_{m}")
                for k in range(ksub):
                    nc.tensor.matmul(
                        pz[:],
                        lhsT=w_z[:, k, m * P : (m + 1) * P],
                        rhs=rhs_tiles[k][:],
                        start=(k == 0),
                        stop=(k == ksub - 1),
                    )
                for k in range(ksub):
                    nc.tensor.matmul(
                        ph[:],
                        lhsT=w_h[:, k, m * P : (m + 1) * P],
                        rhs=rhs_tiles[k][:],
                        start=(k == 0),
                        stop=(k == ksub - 1),
                    )
                # a = 1 - z = sigmoid(-(pre_z + bz)); z = sigmoid(pre_z + bz)
                a_t = sb.tile(
                    [P, tcc], BF16, tag=f"a{ltag}", bufs=4, name=f"a{ltag}_{i}_{m}"
                )
                nc.scalar.activation(
                    a_t[:], pz[:], ACTF.Sigmoid, bias=nbz[:, m : m + 1], scale=-1.0
                )
                z_t = sb.tile(
                    [P, tcc], BF16, tag=f"z{ltag}", bufs=4, name=f"z{ltag}_{i}_{m}"
                )
                nc.scalar.activation(
                    z_t[:], pz[:], ACTF.Sigmoid, bias=bz[:, m : m + 1], scale=1.0
                )
                # b = (pre_h + bh) * z
                b_t = sb.tile(
                    [P, tcc], BF16, tag=f"b{ltag}", bufs=4, name=f"b{ltag}_{i}_{m}"
                )
                nc.vector.scalar_tensor_tensor(
                    b_t[:], ph[:], bh[:, m : m + 1], z_t[:], op0=ALU.add, op1=ALU.mult
                )
                # h_t = a_t * h_{t-1} + b_t along the time (free) axis
                h_t = sb.tile(
                    [P, tcc], BF16, tag=f"h{ltag}_{m}", bufs=3, name=f"h{ltag}_{i}_{m}"
                )
                init = 0.0 if carry[m] is None else carry[m][:, tcc - 1 : tcc]
                nc.vector.tensor_tensor_scan(
                    h_t[:], a_t[:], b_t[:], init, op0=ALU.mult, op1=ALU.add
                )
                carry[m] = h_t
                outs.append(h_t)
            out_tiles[i] = outs

        def emit_FC(i):
            h1 = h1_tiles.pop(i)
            for tt in range(tcc // P):
                yp = psum.tile([P, DOUT], F32, tag="psum", name=f"yp_{i}_{tt}")
                for j in range(hsub):
                    nc.tensor.matmul(
                        yp[:],
                        lhsT=h1[j][:, tt * P : (tt + 1) * P],
                        rhs=wfc_sb[:, j, :],
                        start=(j == 0),
                        stop=(j == hsub - 1),
                    )
                y_sb = sb.tile([P, DOUT], F32, tag="y", bufs=4, name=f"y_{i}_{tt}")
                nc.vector.tensor_tensor(y_sb[:], yp[:], bfc_sb[:], ALU.add)
                t0 = i * tcc + tt * P
                nc.sync.dma_start(y_d[t0 : t0 + P, :], y_sb[:])

        # Software-pipelined chunk loop (stages offset on the PE stream).
        for i in range(nchunk + 2):
            if i < nchunk:
                emit_layer(
                    i, wz0_sb, wh0_sb, bz0_sb, nbz0_sb, bh0_sb,
                    xT_tiles.pop(i), carry0, h0_tiles, "0",
                )
            if i + 1 < nchunk:
                emit_T(i + 1)
            if 1 <= i <= nchunk:
                emit_layer(
                    i - 1, wz1_sb, wh1_sb, bz1_sb, nbz1_sb, bh1_sb,
                    h0_tiles.pop(i - 1), carry1, h1_tiles, "1",
                )
            if 2 <= i <= nchunk + 1:
                emit_FC(i - 2)

    nc.compile()
    return nc


_NC_CACHE = {}


def _get_nc(t_total=T, tcc=TC):
    key = (t_total, tcc)
    if key not in _NC_CACHE:
        _NC_CACHE[key] = build(t_total, tcc)
    return _NC_CACHE[key]


def run(inputs, trace=False, **spmd_kwargs):
    """Run the SPMD kernel on all 8 cores. Returns (y[B,T,DOUT], results)."""
    x = np.asarray(inputs["x"], dtype=np.float32)
    assert x.shape == (B, T, DIN), x.shape
    x_bf = np.ascontiguousarray(x.astype(ml_dtypes.bfloat16))
    shared = {}
    for name in GEMM_W:
        shared[name] = np.ascontiguousarray(
            np.asarray(inputs[name], dtype=np.float32).astype(ml_dtypes.bfloat16)
        )
    for name in BIASES:
        shared[name] = np.ascontiguousarray(np.asarray(inputs[name], dtype=np.float32))
    nc = _get_nc()
    in_maps = [dict(shared, x=x_bf[c]) for c in range(B)]
    res = bass_utils.run_bass_kernel_spmd(
        nc, in_maps, core_ids=list(range(B)), trace=trace, **spmd_kwargs
    )
    y = np.stack([r["y"] for r in res.results], axis=0).astype(np.float32)
    return y, res


def kernel(**inputs) -> np.ndarray:
    y, _ = run(inputs)
    return y
